# revision 2
# baseline (speedup 1.0000x reference)
"""Trainium2 Bass kernel computing out = x * exp(diagonal).

x: (8192, 4096) float32, diagonal: (4096,) float32.
Data-parallel across 8 NeuronCores: each core handles 1024 rows of x;
the 4096-float diagonal is replicated to every core.

Per-core program (pure streaming, memory-bound). TRN2 compute/DMA
instructions only carry ONE sync-wait command, and Tile has 8 HWDGE
completion-sem lanes, so the program is shaped to need at most one wait
per instruction and at most 8 HWDGE DMAs (no lane reuse):

  1. exp(diagonal) broadcast tile [128, 4096] built via a stride-0
     SWDGE DMA from DRAM (separate sem lanes) + ACT Exp.
  2. A 1-element DVE copy observes the Exp so later muls don't need a
     second wait on it.
  3. x streams through 4 fresh [128, 8192] SBUF tiles (half the 16 MiB
     shard resident at once, no slot reuse => no WAR waits):
     HWDGE load on SP -> in-place DVE multiply (the exp-vector operand
     is free-dim-broadcast 2x) -> HWDGE store on ACT.
"""

import numpy as np

BATCH, FEAT = 8192, 4096
N_CORES = 8
ROWS = BATCH // N_CORES   # 1024 rows per core
P = 128                   # SBUF partitions
FOLD = 2                  # row-blocks folded into one tile's free dim
N_TILES = ROWS // (P * FOLD)  # 4 tiles of [128, FOLD*4096] per core

_CACHE = {}


def build_nc(rows=ROWS, feat=FEAT, fold=FOLD):
    import concourse.bacc as bacc
    import concourse.mybir as mybir
    from concourse import tile

    # Bacc (not plain Bass): its compile() pass splits multi-sem waits into
    # EventSemaphore chains -- TRN2 instructions carry at most one wait.
    nc = bacc.Bacc("TRN2", target_bir_lowering=False, debug=False)
    x = nc.dram_tensor("x", (rows, feat), mybir.dt.float32, kind="ExternalInput").ap()
    d = nc.dram_tensor("d", (feat,), mybir.dt.float32, kind="ExternalInput").ap()
    out = nc.dram_tensor(
        "out", (rows, feat), mybir.dt.float32, kind="ExternalOutput"
    ).ap()

    n_tiles = rows // (P * fold)
    x_t = x.rearrange("(s n p) m -> s p n m", p=P, n=fold)
    o_t = out.rearrange("(s n p) m -> s p n m", p=P, n=fold)
    d_row = d.rearrange("(r c) -> r c", r=1)

    with tile.TileContext(nc) as tc:
        with (
            tc.tile_pool(name="const", bufs=1) as cpool,
            tc.tile_pool(name="io", bufs=n_tiles) as iopool,
        ):
            expd = cpool.tile([P, feat], mybir.dt.float32)
            nc.gpsimd.dma_start(expd[:], d_row.to_broadcast((P, feat)))
            nc.scalar.activation(expd[:], expd[:], mybir.ActivationFunctionType.Exp)
            # DVE observer: absorbs the wait on the Exp so the muls below
            # carry exactly one wait (their own load DMA).
            scratch = cpool.tile([1, 1], mybir.dt.float32)
            nc.vector.tensor_copy(scratch[:], expd[0:1, 0:1])
            # exp vector broadcast FOLD x along the free dim (stride 0)
            expd_b = expd[:].rearrange("p (o m) -> p o m", o=1).to_broadcast(
                (P, fold, feat)
            )

            tiles = []
            for i in range(n_tiles):
                t = iopool.tile([P, fold * feat], mybir.dt.float32)
                t3 = t.rearrange("p (n m) -> p n m", n=fold)
                nc.sync.dma_start(t3, x_t[i])
                tiles.append(t3)
            for i, t3 in enumerate(tiles):
                nc.vector.tensor_mul(t3, t3, expd_b)
                nc.scalar.dma_start(o_t[i], t3)
    nc.finalize()
    return nc


def make_in_maps(x, d):
    return [{"x": x[c * ROWS : (c + 1) * ROWS], "d": d} for c in range(N_CORES)]


def assemble_out(results):
    return np.concatenate([r["out"] for r in results], axis=0)


def kernel(x, diagonal):
    from concourse.bass_utils import run_bass_kernel_spmd

    if "nc" not in _CACHE:
        _CACHE["nc"] = build_nc()
    nc = _CACHE["nc"]

    x = np.ascontiguousarray(x, dtype=np.float32)
    d = np.ascontiguousarray(diagonal, dtype=np.float32)
    in_maps = make_in_maps(x, d)
    res = run_bass_kernel_spmd(nc, in_maps, core_ids=list(range(N_CORES)))
    return assemble_out(res.results)



# revision 4
# speedup vs baseline: 1.0855x; 1.0855x over previous
"""Trainium2 Bass kernel computing out = x * exp(diagonal).

x: (8192, 4096) float32, diagonal: (4096,) float32.
Data-parallel across 8 NeuronCores: each core handles 1024 rows of x;
exp(diagonal) is precomputed on host (4096 floats, negligible) and
pre-broadcast to a (128, 4096) block loaded once per core.

Per-core program (pure streaming, memory-bound):

  1. expd [128, 4096] loaded via HWDGE on the PE queue (its own queue,
     so it lands ~8us in, off the critical path of the x stream).
  2. 1-element observer copies on DVE and Pool absorb the expd wait so
     every multiply carries exactly one wait (its own load DMA).
  3. x streams through 8 fresh [128, 4096] SBUF tiles (16 MiB resident,
     no slot reuse => no WAR waits): HWDGE load on SP queue -> in-place
     multiply alternating DVE/Pool (plain 2D operands) -> HWDGE store
     on ACT queue. Loads and stores run on separate queues and overlap
     for nearly the whole kernel.
"""

import numpy as np

BATCH, FEAT = 8192, 4096
N_CORES = 8
ROWS = BATCH // N_CORES   # 1024 rows per core
P = 128                   # SBUF partitions
N_TILES = ROWS // P       # 8 tiles of [128, 4096] per core

_CACHE = {}


def build_nc(rows=ROWS, feat=FEAT):
    import concourse.bacc as bacc
    import concourse.mybir as mybir
    from concourse import tile

    # Bacc (not plain Bass): its compile() pass splits multi-sem waits into
    # EventSemaphore chains -- TRN2 instructions carry at most one wait.
    nc = bacc.Bacc("TRN2", target_bir_lowering=False, debug=False)
    x = nc.dram_tensor("x", (rows, feat), mybir.dt.float32, kind="ExternalInput").ap()
    dexpb = nc.dram_tensor(
        "dexpb", (P, feat), mybir.dt.float32, kind="ExternalInput"
    ).ap()
    out = nc.dram_tensor(
        "out", (rows, feat), mybir.dt.float32, kind="ExternalOutput"
    ).ap()

    n_tiles = rows // P
    x_t = x.rearrange("(s p) m -> s p m", p=P)
    o_t = out.rearrange("(s p) m -> s p m", p=P)

    with tile.TileContext(nc) as tc:
        with (
            tc.tile_pool(name="const", bufs=1) as cpool,
            tc.tile_pool(name="io", bufs=n_tiles) as iopool,
        ):
            expd = cpool.tile([P, feat], mybir.dt.float32)
            # On the ACT (store) queue: empty at kernel start, so this runs
            # immediately, in parallel with the x loads on the SP queue.
            nc.scalar.dma_start(expd[:], dexpb)
            # Observers: absorb the wait on the expd load so the muls below
            # carry exactly one wait (their own load DMA).
            scr_v = cpool.tile([1, 1], mybir.dt.float32)
            scr_g = cpool.tile([1, 1], mybir.dt.float32)
            nc.vector.tensor_copy(scr_v[:], expd[0:1, 0:1])
            nc.gpsimd.tensor_copy(scr_g[:], expd[0:1, 0:1])

            tiles = []
            for i in range(n_tiles):
                t = iopool.tile([P, feat], mybir.dt.float32)
                nc.sync.dma_start(t[:], x_t[i])
                tiles.append(t)
            for i, t in enumerate(tiles):
                eng = nc.vector if i % 2 == 0 else nc.gpsimd
                eng.tensor_mul(t[:], t[:], expd[:])
                nc.scalar.dma_start(o_t[i], t[:])
    nc.finalize()
    return nc


def make_in_maps(x, d):
    dexp = np.exp(d, dtype=np.float32)
    dexpb = np.ascontiguousarray(np.broadcast_to(dexp, (P, FEAT)))
    return [
        {"x": x[c * ROWS : (c + 1) * ROWS], "dexpb": dexpb} for c in range(N_CORES)
    ]


def assemble_out(results):
    return np.concatenate([r["out"] for r in results], axis=0)


def kernel(x, diagonal):
    from concourse.bass_utils import run_bass_kernel_spmd

    if "nc" not in _CACHE:
        _CACHE["nc"] = build_nc()
    nc = _CACHE["nc"]

    x = np.ascontiguousarray(x, dtype=np.float32)
    d = np.ascontiguousarray(diagonal, dtype=np.float32)
    in_maps = make_in_maps(x, d)
    res = run_bass_kernel_spmd(nc, in_maps, core_ids=list(range(N_CORES)))
    return assemble_out(res.results)


# revision 5
# speedup vs baseline: 1.1175x; 1.0294x over previous
"""Trainium2 Bass kernel computing out = x * exp(diagonal).

x: (8192, 4096) float32, diagonal: (4096,) float32.
Data-parallel across 8 NeuronCores: each core handles 1024 rows of x;
exp(diagonal) is precomputed on host (4096 floats, negligible) and
loaded once per core as a 16 KiB row.

Per-core program (pure streaming, memory-bound; per-core HBM share is
~360 GB/s so the floor is ~32 MiB / 360 GB/s ~= 93 us):

  1. dexp row [1, 4096] loaded via HWDGE on the ACT queue (empty at
     start, lands ~8 us in), replicated to [128, 4096] on-chip by the
     Pool engine's partition_broadcast -- no 2 MiB HBM broadcast read.
  2. A 1-element DVE observer copy absorbs the expd dependency so every
     multiply carries exactly one wait (its own load DMA).
  3. x streams through fresh SBUF tiles (16 MiB resident, no slot
     reuse => no WAR waits): HWDGE load on SP queue -> in-place DVE
     multiply -> HWDGE store on ACT queue. Loads and stores overlap for
     nearly the whole kernel. The last row-block is split into two
     column halves so the final mul+store drain is short.
"""

import numpy as np

BATCH, FEAT = 8192, 4096
N_CORES = 8
ROWS = BATCH // N_CORES   # 1024 rows per core
P = 128                   # SBUF partitions
N_TILES = ROWS // P       # 8 row-blocks of [128, 4096] per core

_CACHE = {}


def build_nc(rows=ROWS, feat=FEAT):
    import concourse.bacc as bacc
    import concourse.mybir as mybir
    from concourse import tile

    # Bacc (not plain Bass): its compile() pass splits multi-sem waits into
    # EventSemaphore chains -- TRN2 instructions carry at most one wait.
    nc = bacc.Bacc("TRN2", target_bir_lowering=False, debug=False)
    x = nc.dram_tensor("x", (rows, feat), mybir.dt.float32, kind="ExternalInput").ap()
    dexp = nc.dram_tensor(
        "dexp", (feat,), mybir.dt.float32, kind="ExternalInput"
    ).ap()
    out = nc.dram_tensor(
        "out", (rows, feat), mybir.dt.float32, kind="ExternalOutput"
    ).ap()

    n_tiles = rows // P
    x_t = x.rearrange("(s p) m -> s p m", p=P)
    o_t = out.rearrange("(s p) m -> s p m", p=P)
    d_row = dexp.rearrange("(r c) -> r c", r=1)

    with tile.TileContext(nc) as tc:
        with (
            tc.tile_pool(name="const", bufs=1) as cpool,
            tc.tile_pool(name="io", bufs=n_tiles) as iopool,
        ):
            row = cpool.tile([1, feat], mybir.dt.float32)
            # ACT (store) queue is empty at kernel start: lands immediately.
            nc.scalar.dma_start(row[:], d_row)
            expd = cpool.tile([P, feat], mybir.dt.float32)
            nc.gpsimd.partition_broadcast(expd[:], row[:])
            # Observer: absorbs the wait on the broadcast so the muls below
            # carry exactly one wait (their own load DMA).
            scr_v = cpool.tile([1, 1], mybir.dt.float32)
            nc.vector.tensor_copy(scr_v[:], expd[0:1, 0:1])

            half = feat // 2
            tiles = []
            for i in range(n_tiles):
                t = iopool.tile([P, feat], mybir.dt.float32)
                if i < n_tiles - 1:
                    nc.sync.dma_start(t[:], x_t[i])
                else:
                    # final block in two halves: short drain at the tail
                    nc.sync.dma_start(t[:, 0:half], x_t[i][:, 0:half])
                    nc.sync.dma_start(t[:, half:feat], x_t[i][:, half:feat])
                tiles.append(t)
            for i, t in enumerate(tiles):
                if i < n_tiles - 1:
                    nc.vector.tensor_mul(t[:], t[:], expd[:])
                    nc.scalar.dma_start(o_t[i], t[:])
                else:
                    nc.vector.tensor_mul(
                        t[:, 0:half], t[:, 0:half], expd[:, 0:half]
                    )
                    nc.scalar.dma_start(o_t[i][:, 0:half], t[:, 0:half])
                    nc.vector.tensor_mul(
                        t[:, half:feat], t[:, half:feat], expd[:, half:feat]
                    )
                    nc.scalar.dma_start(o_t[i][:, half:feat], t[:, half:feat])
    nc.finalize()
    return nc


def make_in_maps(x, d):
    dexp = np.exp(d, dtype=np.float32)
    return [{"x": x[c * ROWS : (c + 1) * ROWS], "dexp": dexp} for c in range(N_CORES)]


def assemble_out(results):
    return np.concatenate([r["out"] for r in results], axis=0)


def kernel(x, diagonal):
    from concourse.bass_utils import run_bass_kernel_spmd

    if "nc" not in _CACHE:
        _CACHE["nc"] = build_nc()
    nc = _CACHE["nc"]

    x = np.ascontiguousarray(x, dtype=np.float32)
    d = np.ascontiguousarray(diagonal, dtype=np.float32)
    in_maps = make_in_maps(x, d)
    res = run_bass_kernel_spmd(nc, in_maps, core_ids=list(range(N_CORES)))
    return assemble_out(res.results)
